# revision 3
# baseline (speedup 1.0000x reference)
"""Trainium2 Bass kernel for a GNN node-aggregator.

Math (reference):
    out[n] = sum_k Linear(concat(v[n], u[k, n]))          with W = [Wv | Wu]
           = (sum_k u[k]) @ Wu.T  +  K * (v @ Wv.T)  +  K * b

The sum over neighbors commutes with the linear layer, so the kernel
streams the big [K, N, D] neighbors tensor once (memory bound),
accumulates the K-sum on the Vector engine, transposes 128x128 node
blocks on the Tensor engine (identity matmul), and finishes with two
small matmuls against host-preprocessed weights plus a bias add.

Distribution: nodes are sharded across 8 NeuronCores.  Every core runs
the same program over 6272 = 49*128 nodes; the core slices overlap
slightly (50000 is not divisible by 8*128) and the host gather keeps
each core's owned rows only.
"""

import numpy as np

N_NODES = 50000
K_NB = 32
D = 128  # in features
O = 128  # out features
P = 128  # SBUF partitions

N_CORES = 8
QB = 49                # 128-node blocks per core
NC_NODES = P * QB      # 6272 nodes per core (overlapped shard)
CHUNK_Q = 7            # q-blocks per pipelined chunk
N_CHUNKS = QB // CHUNK_Q


def _core_starts():
    step = N_NODES // N_CORES
    return [min(c * step, N_NODES - NC_NODES) for c in range(N_CORES)]


def _build(k_nb=K_NB, qb=QB, chunk_q=CHUNK_Q, repeats=1, k_bufs=6):
    """Build the per-core Bass program (SPMD: same NEFF on all cores)."""
    import concourse.mybir as mybir
    import concourse.tile as tile
    from concourse import bacc

    f32 = mybir.dt.float32
    nc_nodes = P * qb
    n_chunks = qb // chunk_q
    assert qb % chunk_q == 0
    cw = chunk_q * D  # chunk width in free elements

    nc = bacc.Bacc(trn_type="TRN2", name="node_aggregator")
    nbr = nc.dram_tensor("nbr", [k_nb, nc_nodes, D], f32, kind="ExternalInput")
    vin = nc.dram_tensor("vin", [nc_nodes, D], f32, kind="ExternalInput")
    wut = nc.dram_tensor("wut", [D, O], f32, kind="ExternalInput")    # Wu.T
    wvtk = nc.dram_tensor("wvtk", [D, O], f32, kind="ExternalInput")  # K * Wv.T
    bbc = nc.dram_tensor("bbc", [P, O], f32, kind="ExternalInput")    # K*b rows
    iden = nc.dram_tensor("iden", [P, P], f32, kind="ExternalInput")
    out = nc.dram_tensor("out", [nc_nodes, O], f32, kind="ExternalOutput")

    # Partition p holds nodes [qb*p, qb*p + qb): contiguous 49*512B per
    # partition in DRAM, so every chunk DMA is 128 x 3.5KB contiguous runs.
    nbr_r = nbr[:].rearrange("k (p q) d -> k p (q d)", p=P)
    v_r = vin[:].rearrange("(p q) d -> p (q d)", p=P)
    out_r = out[:].rearrange("(p q) o -> p (q o)", p=P)

    with tile.TileContext(nc) as tc:
        with (
            tc.tile_pool(name="cpool", bufs=1) as cpool,
            tc.tile_pool(name="kpool", bufs=k_bufs) as kpool,
            tc.tile_pool(name="apool", bufs=2) as apool,
            tc.tile_pool(name="vpool", bufs=2) as vpool,
            tc.tile_pool(name="opool", bufs=2) as opool,
            tc.tile_pool(name="bpool", bufs=3) as bpool,
            tc.tile_pool(name="ptp", bufs=2, space="PSUM") as ptp,
            tc.tile_pool(name="pop", bufs=2, space="PSUM") as pop,
        ):
            wut_t = cpool.tile([D, O], f32)
            nc.sync.dma_start(wut_t[:], wut[:])
            wvtk_t = cpool.tile([D, O], f32)
            nc.sync.dma_start(wvtk_t[:], wvtk[:])
            bbc_t = cpool.tile([P, O], f32)
            nc.sync.dma_start(bbc_t[:], bbc[:])
            iden_t = cpool.tile([P, P], f32)
            nc.sync.dma_start(iden_t[:], iden[:])

            for _ in range(repeats):
                for c in range(n_chunks):
                    cs = slice(c * cw, (c + 1) * cw)
                    # K-sum of this chunk's neighbor slabs, in place on S.
                    S = apool.tile([P, cw], f32, tag="S")
                    nc.sync.dma_start(S[:], nbr_r[0, :, cs])
                    for k in range(1, k_nb):
                        kt = kpool.tile([P, cw], f32, tag="kt")
                        nc.sync.dma_start(kt[:], nbr_r[k, :, cs])
                        nc.vector.tensor_add(out=S[:], in0=S[:], in1=kt[:])
                    vt = vpool.tile([P, cw], f32, tag="vt")
                    nc.sync.dma_start(vt[:], v_r[:, cs])
                    ot = opool.tile([P, cw], f32, tag="ot")
                    for qq in range(chunk_q):
                        qs = slice(qq * D, (qq + 1) * D)
                        # PE transpose S block and v block to [d, n] layout.
                        pt1 = ptp.tile([D, P], f32, tag="pt1")
                        nc.tensor.transpose(pt1[:], S[:, qs], iden_t[:])
                        st = bpool.tile([D, P], f32, tag="st")
                        nc.any.tensor_copy(out=st[:], in_=pt1[:])
                        pt2 = ptp.tile([D, P], f32, tag="pt2")
                        nc.tensor.transpose(pt2[:], vt[:, qs], iden_t[:])
                        vq = bpool.tile([D, P], f32, tag="vq")
                        nc.any.tensor_copy(out=vq[:], in_=pt2[:])
                        # out_block = S_blk @ Wu.T + v_blk @ (K Wv).T (+ K b)
                        op = pop.tile([P, O], f32, tag="op")
                        nc.tensor.matmul(
                            op[:], lhsT=st[:], rhs=wut_t[:], start=True, stop=False
                        )
                        nc.tensor.matmul(
                            op[:], lhsT=vq[:], rhs=wvtk_t[:], start=False, stop=True
                        )
                        nc.vector.tensor_add(out=ot[:, qs], in0=op[:], in1=bbc_t[:])
                    nc.sync.dma_start(out_r[:, cs], ot[:])
    nc.compile()
    return nc


def _prep_weights(W, b):
    Wv = W[:, :D]
    Wu = W[:, D:]
    wut = np.ascontiguousarray(Wu.T, dtype=np.float32)
    wvtk = np.ascontiguousarray(Wv.T * np.float32(K_NB), dtype=np.float32)
    bbc = np.ascontiguousarray(
        np.broadcast_to((np.float32(K_NB) * b).astype(np.float32), (P, O))
    )
    iden = np.eye(P, dtype=np.float32)
    return wut, wvtk, bbc, iden


def kernel(v, neighbors, W, b):
    from concourse.bass_utils import run_bass_kernel_spmd

    v = np.asarray(v, dtype=np.float32)
    neighbors = np.asarray(neighbors, dtype=np.float32)
    W = np.asarray(W, dtype=np.float32)
    b = np.asarray(b, dtype=np.float32)

    wut, wvtk, bbc, iden = _prep_weights(W, b)
    nc = _build()
    starts = _core_starts()
    in_maps = [
        {
            "nbr": np.ascontiguousarray(neighbors[:, s : s + NC_NODES, :]),
            "vin": np.ascontiguousarray(v[s : s + NC_NODES]),
            "wut": wut,
            "wvtk": wvtk,
            "bbc": bbc,
            "iden": iden,
        }
        for s in starts
    ]
    res = run_bass_kernel_spmd(nc, in_maps, core_ids=list(range(N_CORES)))

    out = np.empty((N_NODES, O), dtype=np.float32)
    step = N_NODES // N_CORES
    for c, s in enumerate(starts):
        own_lo = c * step
        own_hi = N_NODES if c == N_CORES - 1 else (c + 1) * step
        r = res.results[c]["out"]
        out[own_lo:own_hi] = r[own_lo - s : own_hi - s]
    return out
